# revision 6
# baseline (speedup 1.0000x reference)
"""GCN (3x spmm + linear) Trainium2 Bass kernel for nn_GCNModel_75557064671960.

Model: out = A(A(A·x·W1 + b1)·W2 + b2)·W3 + b3, where A is a 50000x50000
sparse matrix with 800k weighted edges (duplicate edges sum).

Strategy (8 NeuronCores, SPMD via run_bass_kernel_spmd):
  - Nodes are dst-sharded: core c owns dst rows [c*6250, (c+1)*6250).
  - Each layer computes z = h·W on local node tiles (TensorE), AllGathers the
    z table (bf16) so every core sees all 50176 (padded) rows, then does the
    sparse aggregation locally:
      * per-edge rows of z are fetched with bulk SWDGE dma_gather (int16
        indices; the table is split in two 25088-row halves so indices fit),
      * segment-sum is a one-hot matmul: S[e, p] = val_e * (dst_rel_e == p)
        built on VectorE (is_equal vs an iota tile), accumulated on TensorE
        into PSUM per 128-node dst tile.
  - Edges are pre-sorted by (core, dst tile, table half) on the host and
    padded per (tile, half) to 9 chunks of 128 edge slots (pad slots gather
    row 0 with val 0, contributing nothing).
  - Layer 3 runs in fp32 with the flipped matmul orientation so the output
    lands directly in [node, 64] layout.

If anything about the input violates the precomputed capacity assumptions
(or no neuron devices are present), falls back to an exact scipy host path.
"""

import os

import numpy as np

# ---------------------------------------------------------------- constants
N_NODES = 50000
N_EDGES = 800000
IN_DIM, HID_DIM, OUT_DIM = 128, 128, 64
NC_ = 8
PER = N_NODES // NC_          # 6250 nodes per core
T = 49                        # dst tiles of 128 nodes per core (49*128 = 6272)
PADN = T * 128                # 6272 padded local rows
TABLE = NC_ * PADN            # 50176 global table rows
LO = TABLE // 2               # 25088: first half of the table (int16-safe)
CPH = 9                       # chunks (of 128 edge slots) per (tile, half)
CAP = CPH * 128               # 1152 slots per (tile, half)
SLOTS_R = T * CAP             # 56448 slots per region (half) per core
COLS_R = SLOTS_R // 128       # 441 gather cols per region
GCALL = 1024                  # max descriptors per dma_gather (ucode ring)
GCOLS = GCALL // 128          # 8 msg cols per full gather call
NPIECE = (COLS_R + GCOLS - 1) // GCOLS  # 56 gather calls per region per layer

# exec time of the last device run (ns), set when GCN_TRACE=1
LAST_EXEC_NS = None
LAST_RESULTS = None


# ---------------------------------------------------------------- host path
def _host_kernel(x, adj_indices, adj_values, W1, b1, W2, b2, W3, b3):
    dst = np.asarray(adj_indices[0], dtype=np.int64)
    src = np.asarray(adj_indices[1], dtype=np.int64)
    vals = np.asarray(adj_values, dtype=np.float32)
    from scipy.sparse import csr_matrix

    A = csr_matrix((vals, (dst, src)), shape=(N_NODES, N_NODES))
    h = np.asarray(A @ np.asarray(x, dtype=np.float32)) @ W1 + b1
    h = np.asarray(A @ h) @ W2 + b2
    return (np.asarray(A @ h) @ W3 + b3).astype(np.float32)


# ----------------------------------------------------------- host-side prep
def _prep_edges(adj_indices, adj_values):
    """Sort/pad edges into the per-core slot layout.

    Returns per-core packed arrays or None if capacities are exceeded:
      idx  [8, 2, 128, SLOTS_R//16] int16  (dma_gather wrapped layout)
      drel [8, 2, 128, COLS_R] float32     (dst one-hot position, 0..127)
      val  [8, 2, 128, COLS_R] float32
    """
    dst = np.asarray(adj_indices[0], dtype=np.int64)
    src = np.asarray(adj_indices[1], dtype=np.int64)
    vals = np.asarray(adj_values, dtype=np.float32)
    if dst.min() < 0 or dst.max() >= N_NODES or src.min() < 0 or src.max() >= N_NODES:
        return None

    core = dst // PER
    dl = dst - core * PER
    tl = dl >> 7
    drel = (dl & 127).astype(np.float32)
    core_s = src // PER
    tp = core_s * PADN + (src - core_s * PER)
    hi = (tp >= LO).astype(np.int64)
    idxv = np.where(hi == 1, tp - LO, tp).astype(np.int64)

    key = (core * T + tl) * 2 + hi
    counts = np.bincount(key, minlength=NC_ * T * 2)
    if counts.max() > CAP:
        return None
    order = np.argsort(key, kind="stable")
    starts = np.zeros_like(counts)
    starts[1:] = np.cumsum(counts)[:-1]
    g = key[order]
    rank = np.arange(len(dst)) - starts[g]
    c_ = g // (T * 2)
    rem = g - c_ * (T * 2)
    t_ = rem >> 1
    h_ = rem & 1
    slot = t_ * CAP + rank

    idx_arr = np.zeros((NC_, 2, SLOTS_R), np.int16)
    dr_arr = np.zeros((NC_, 2, SLOTS_R), np.float32)
    va_arr = np.zeros((NC_, 2, SLOTS_R), np.float32)
    idx_arr[c_, h_, slot] = idxv[order].astype(np.int16)
    dr_arr[c_, h_, slot] = drel[order]
    va_arr[c_, h_, slot] = vals[order]

    # dma_gather index layout: slot i -> [partition i%16 (replicated to 128), col i//16]
    idx_w = idx_arr.reshape(NC_, 2, SLOTS_R // 16, 16).transpose(0, 1, 3, 2)
    idx_buf = np.tile(idx_w, (1, 1, 8, 1))  # [8, 2, 128, 3528]
    # drel/val layout: slot i -> [partition i%128, col i//128]
    dr_buf = np.ascontiguousarray(dr_arr.reshape(NC_, 2, COLS_R, 128).transpose(0, 1, 3, 2))
    va_buf = np.ascontiguousarray(va_arr.reshape(NC_, 2, COLS_R, 128).transpose(0, 1, 3, 2))
    return np.ascontiguousarray(idx_buf), dr_buf, va_buf


# ------------------------------------------------------------ device kernel
def _build_nc():
    import concourse.bacc as bacc
    import concourse.bass as bass
    import concourse.mybir as mybir
    import concourse.tile as tile

    BF16 = mybir.dt.bfloat16
    F32 = mybir.dt.float32
    I16 = mybir.dt.int16
    ts = bass.ts

    nc = bacc.Bacc("TRN2", target_bir_lowering=False, debug=False,
                   enable_asserts=False, num_devices=NC_)

    xT_in = nc.dram_tensor("xT", [128, PADN], BF16, kind="ExternalInput")
    idxA_in = nc.dram_tensor("idxA", [128, SLOTS_R // 16], I16, kind="ExternalInput")
    idxB_in = nc.dram_tensor("idxB", [128, SLOTS_R // 16], I16, kind="ExternalInput")
    drA_in = nc.dram_tensor("drA", [128, COLS_R], BF16, kind="ExternalInput")
    drB_in = nc.dram_tensor("drB", [128, COLS_R], BF16, kind="ExternalInput")
    vaA_in = nc.dram_tensor("vaA", [128, COLS_R], BF16, kind="ExternalInput")
    vaB_in = nc.dram_tensor("vaB", [128, COLS_R], BF16, kind="ExternalInput")
    drA32_in = nc.dram_tensor("drA32", [128, COLS_R], F32, kind="ExternalInput")
    drB32_in = nc.dram_tensor("drB32", [128, COLS_R], F32, kind="ExternalInput")
    vaA32_in = nc.dram_tensor("vaA32", [128, COLS_R], F32, kind="ExternalInput")
    vaB32_in = nc.dram_tensor("vaB32", [128, COLS_R], F32, kind="ExternalInput")
    w1_in = nc.dram_tensor("w1", [128, 128], BF16, kind="ExternalInput")
    w2_in = nc.dram_tensor("w2", [128, 128], BF16, kind="ExternalInput")
    w3_in = nc.dram_tensor("w3", [128, OUT_DIM], BF16, kind="ExternalInput")
    b1_in = nc.dram_tensor("b1", [128, 1], F32, kind="ExternalInput")
    b2_in = nc.dram_tensor("b2", [128, 1], F32, kind="ExternalInput")
    b3b_in = nc.dram_tensor("b3b", [128, OUT_DIM], F32, kind="ExternalInput")
    iota_in = nc.dram_tensor("iota", [128, 128], BF16, kind="ExternalInput")
    iota32_in = nc.dram_tensor("iota32", [128, 128], F32, kind="ExternalInput")
    out = nc.dram_tensor("out", [PADN, OUT_DIM], F32, kind="ExternalOutput")

    groups = [list(range(NC_))]

    with tile.TileContext(nc) as tc:
        with (
            tc.tile_pool(name="const", bufs=1) as cp,
            tc.tile_pool(name="work", bufs=2) as wp,
            tc.tile_pool(name="ms", bufs=4) as mp,
            tc.tile_pool(name="psum", bufs=2, space="PSUM") as pp,
            tc.tile_pool(name="dram", bufs=1, space="DRAM") as dp,
        ):
            def load(name, inp, shape, dt):
                t = cp.tile(shape, dt, tag=name)
                nc.sync.dma_start(out=t[:], in_=inp[:, :])
                return t

            xT = load("xT", xT_in, [128, PADN], BF16)
            idxA = load("idxA", idxA_in, [128, SLOTS_R // 16], I16)
            idxB = load("idxB", idxB_in, [128, SLOTS_R // 16], I16)
            drA = load("drA", drA_in, [128, COLS_R], BF16)
            drB = load("drB", drB_in, [128, COLS_R], BF16)
            vaA = load("vaA", vaA_in, [128, COLS_R], BF16)
            vaB = load("vaB", vaB_in, [128, COLS_R], BF16)
            drA32 = load("drA32", drA32_in, [128, COLS_R], F32)
            drB32 = load("drB32", drB32_in, [128, COLS_R], F32)
            vaA32 = load("vaA32", vaA32_in, [128, COLS_R], F32)
            vaB32 = load("vaB32", vaB32_in, [128, COLS_R], F32)
            w1 = load("w1", w1_in, [128, 128], BF16)
            w2 = load("w2", w2_in, [128, 128], BF16)
            w3 = load("w3", w3_in, [128, OUT_DIM], BF16)
            b1 = load("b1", b1_in, [128, 1], F32)
            b2 = load("b2", b2_in, [128, 1], F32)
            b3b = load("b3b", b3b_in, [128, OUT_DIM], F32)
            iota = load("iota", iota_in, [128, 128], BF16)
            iota32 = load("iota32", iota32_in, [128, 128], F32)

            z1l = dp.tile([PADN, 128], BF16)
            zf1 = dp.tile([TABLE, 128], BF16, addr_space="Shared")
            z2l = dp.tile([PADN, 128], BF16)
            zf2 = dp.tile([TABLE, 128], BF16, addr_space="Shared")
            z3l = dp.tile([PADN, OUT_DIM], F32)
            zf3 = dp.tile([TABLE, OUT_DIM], F32, addr_space="Shared")

            def bc3(ap2d, t, n):
                # [128, COLS_R] slice for tile t -> [128, CPH, n] broadcast
                return ap2d[:, t * CPH:(t + 1) * CPH].to_broadcast([128, CPH, n])

            def iota3(it, n):
                a = it[:]
                return bass.AP(a.tensor, a.offset, [a.ap[0], [0, CPH], [1, n]])

            # ---- dense layer 1: z1 = x @ W1 (per local tile)
            for t in range(T):
                pz = pp.tile([128, 128], F32, tag="pz", space="PSUM")
                nc.tensor.matmul(out=pz[:], lhsT=xT[:, ts(t, 128)], rhs=w1[:],
                                 start=True, stop=True)
                zt = wp.tile([128, 128], BF16, tag="zt")
                nc.scalar.copy(zt[:], pz[:])
                nc.sync.dma_start(out=z1l[ts(t, 128), :], in_=zt[:])
            nc.gpsimd.collective_compute(
                "AllGather", mybir.AluOpType.bypass, replica_groups=groups,
                ins=[z1l.opt()], outs=[zf1.opt()])

            # ---- middle layers: h = A·z + b ; z_next = h @ W_next
            def make_gather_mgr(zf, idx_t, lo_region, elem, dt):
                state = {"next": 0, "pieces": {}}

                def ensure(p):
                    while state["next"] <= p:
                        q = state["next"]
                        cols = min(GCOLS, COLS_R - q * GCOLS)
                        nidx = cols * 128
                        ms = mp.tile([128, cols, elem], dt,
                                     tag="msA" if lo_region else "msB")
                        src_ap = zf[0:LO, :] if lo_region else zf[LO:TABLE, :]
                        nc.gpsimd.dma_gather(
                            out_ap=ms[:], in_ap=src_ap,
                            idxs_ap=idx_t[:, q * (GCALL // 16):
                                          q * (GCALL // 16) + nidx // 16],
                            num_idxs=nidx, num_idxs_reg=nidx,
                            elem_size=elem, queue_num=0)
                        state["pieces"][q] = ms
                        state["next"] = q + 1

                def col(c):
                    return state["pieces"][c // GCOLS][:, c % GCOLS, :]

                def prefetch(t):
                    ensure(min((t * CPH + CPH - 1) // GCOLS, NPIECE - 1))

                return prefetch, col

            def spmm_mid(zf, bias, w_next, z_next):
                pfA, colA = make_gather_mgr(zf, idxA, True, 128, BF16)
                pfB, colB = make_gather_mgr(zf, idxB, False, 128, BF16)
                for t in range(T):
                    pfA(t)
                    pfB(t)
                    SA = wp.tile([128, CPH, 128], BF16, tag="SA")
                    nc.vector.tensor_tensor(out=SA[:], in0=bc3(drA, t, 128),
                                            in1=iota3(iota, 128),
                                            op=mybir.AluOpType.is_equal)
                    nc.vector.tensor_tensor(out=SA[:], in0=SA[:],
                                            in1=bc3(vaA, t, 128),
                                            op=mybir.AluOpType.mult)
                    SB = wp.tile([128, CPH, 128], BF16, tag="SB")
                    nc.vector.tensor_tensor(out=SB[:], in0=bc3(drB, t, 128),
                                            in1=iota3(iota, 128),
                                            op=mybir.AluOpType.is_equal)
                    nc.vector.tensor_tensor(out=SB[:], in0=SB[:],
                                            in1=bc3(vaB, t, 128),
                                            op=mybir.AluOpType.mult)
                    pt = pp.tile([128, 128], F32, tag="pt", space="PSUM")
                    for k in range(CPH):
                        nc.tensor.matmul(out=pt[:], lhsT=colA(t * CPH + k),
                                         rhs=SA[:, k, :], start=(k == 0), stop=False)
                    for k in range(CPH):
                        nc.tensor.matmul(out=pt[:], lhsT=colB(t * CPH + k),
                                         rhs=SB[:, k, :], start=False,
                                         stop=(k == CPH - 1))
                    hT = wp.tile([128, 128], BF16, tag="hT")
                    nc.vector.tensor_tensor(out=hT[:], in0=pt[:],
                                            in1=bias[:, 0:1].to_broadcast([128, 128]),
                                            op=mybir.AluOpType.add)
                    ncols = w_next.shape[-1]
                    pz = pp.tile([128, 128], F32, tag="pz", space="PSUM")
                    nc.tensor.matmul(out=pz[:, :ncols], lhsT=hT[:], rhs=w_next[:],
                                     start=True, stop=True)
                    if ncols == 128:
                        zt = wp.tile([128, 128], BF16, tag="zt")
                        nc.scalar.copy(zt[:], pz[:])
                        nc.sync.dma_start(out=z_next[ts(t, 128), :], in_=zt[:])
                    else:
                        zt = wp.tile([128, ncols], F32, tag="zt32")
                        nc.scalar.copy(zt[:], pz[:, :ncols])
                        nc.sync.dma_start(out=z_next[ts(t, 128), :], in_=zt[:])

            spmm_mid(zf1, b1, w2, z2l)
            nc.gpsimd.collective_compute(
                "AllGather", mybir.AluOpType.bypass, replica_groups=groups,
                ins=[z2l.opt()], outs=[zf2.opt()])
            spmm_mid(zf2, b2, w3, z3l)
            nc.gpsimd.collective_compute(
                "AllGather", mybir.AluOpType.bypass, replica_groups=groups,
                ins=[z3l.opt()], outs=[zf3.opt()])

            # ---- final layer: out = A·z3 + b3 (fp32, direct [node, 64] layout)
            pfA, colA = make_gather_mgr(zf3, idxA, True, OUT_DIM, F32)
            pfB, colB = make_gather_mgr(zf3, idxB, False, OUT_DIM, F32)
            for t in range(T):
                pfA(t)
                pfB(t)
                SA = wp.tile([128, CPH, 128], F32, tag="SA32")
                nc.vector.tensor_tensor(out=SA[:], in0=bc3(drA32, t, 128),
                                        in1=iota3(iota32, 128),
                                        op=mybir.AluOpType.is_equal)
                nc.vector.tensor_tensor(out=SA[:], in0=SA[:],
                                        in1=bc3(vaA32, t, 128),
                                        op=mybir.AluOpType.mult)
                SB = wp.tile([128, CPH, 128], F32, tag="SB32")
                nc.vector.tensor_tensor(out=SB[:], in0=bc3(drB32, t, 128),
                                        in1=iota3(iota32, 128),
                                        op=mybir.AluOpType.is_equal)
                nc.vector.tensor_tensor(out=SB[:], in0=SB[:],
                                        in1=bc3(vaB32, t, 128),
                                        op=mybir.AluOpType.mult)
                pt = pp.tile([128, OUT_DIM], F32, tag="pt", space="PSUM")
                for k in range(CPH):
                    nc.tensor.matmul(out=pt[:], lhsT=SA[:, k, :],
                                     rhs=colA(t * CPH + k), start=(k == 0), stop=False)
                for k in range(CPH):
                    nc.tensor.matmul(out=pt[:], lhsT=SB[:, k, :],
                                     rhs=colB(t * CPH + k), start=False,
                                     stop=(k == CPH - 1))
                ot = wp.tile([128, OUT_DIM], F32, tag="ot")
                nc.vector.tensor_tensor(out=ot[:], in0=pt[:], in1=b3b[:, :],
                                        op=mybir.AluOpType.add)
                nc.sync.dma_start(out=out[ts(t, 128), :], in_=ot[:])

    nc.compile()
    return nc


_NC_CACHE = None


def _device_kernel(x, adj_indices, adj_values, W1, b1, W2, b2, W3, b3):
    global _NC_CACHE, LAST_EXEC_NS, LAST_RESULTS
    import ml_dtypes
    from concourse.bass_utils import run_bass_kernel_spmd

    prep = _prep_edges(adj_indices, adj_values)
    if prep is None:
        raise ValueError("edge layout exceeds padded capacity")
    idx_buf, dr_buf, va_buf = prep

    bf16 = ml_dtypes.bfloat16
    x = np.asarray(x, dtype=np.float32)
    W1 = np.asarray(W1, np.float32)
    W2 = np.asarray(W2, np.float32)
    W3 = np.asarray(W3, np.float32)
    iota_np = np.broadcast_to(np.arange(128, dtype=np.float32), (128, 128))
    b3b_np = np.ascontiguousarray(
        np.broadcast_to(np.asarray(b3, np.float32)[None, :], (128, OUT_DIM)))

    shared = {
        "w1": W1.astype(bf16),
        "w2": W2.astype(bf16),
        "w3": W3.astype(bf16),
        "b1": np.asarray(b1, np.float32).reshape(128, 1),
        "b2": np.asarray(b2, np.float32).reshape(128, 1),
        "b3b": b3b_np,
        "iota": np.ascontiguousarray(iota_np.astype(bf16)),
        "iota32": np.ascontiguousarray(iota_np.astype(np.float32)),
    }
    in_maps = []
    for c in range(NC_):
        xs = np.zeros((128, PADN), np.float32)
        xs[:, :PER] = x[c * PER:(c + 1) * PER].T
        m = {
            "xT": xs.astype(bf16),
            "idxA": idx_buf[c, 0],
            "idxB": idx_buf[c, 1],
            "drA": dr_buf[c, 0].astype(bf16),
            "drB": dr_buf[c, 1].astype(bf16),
            "vaA": va_buf[c, 0].astype(bf16),
            "vaB": va_buf[c, 1].astype(bf16),
            "drA32": dr_buf[c, 0],
            "drB32": dr_buf[c, 1],
            "vaA32": va_buf[c, 0],
            "vaB32": va_buf[c, 1],
        }
        m.update(shared)
        in_maps.append(m)

    if _NC_CACHE is None:
        _NC_CACHE = _build_nc()
    nc = _NC_CACHE

    trace = bool(os.environ.get("GCN_TRACE"))
    res = run_bass_kernel_spmd(nc, in_maps, core_ids=list(range(NC_)), trace=trace)
    LAST_EXEC_NS = res.exec_time_ns
    LAST_RESULTS = res
    outp = np.concatenate([res.results[c]["out"][:PER] for c in range(NC_)], axis=0)
    return np.ascontiguousarray(outp.astype(np.float32))


def kernel(x, adj_indices, adj_values, W1, b1, W2, b2, W3, b3):
    if os.environ.get("GCN_FORCE_HOST"):
        return _host_kernel(x, adj_indices, adj_values, W1, b1, W2, b2, W3, b3)
    try:
        return _device_kernel(x, adj_indices, adj_values, W1, b1, W2, b2, W3, b3)
    except Exception:
        if os.environ.get("GCN_NO_FALLBACK"):
            raise
        return _host_kernel(x, adj_indices, adj_values, W1, b1, W2, b2, W3, b3)


# revision 7
# speedup vs baseline: 3.3979x; 3.3979x over previous
"""GCN (3x spmm + linear) Trainium2 Bass kernel for nn_GCNModel_75557064671960.

Model: out = A(A(A·x·W1 + b1)·W2 + b2)·W3 + b3, where A is a 50000x50000
sparse matrix with 800k weighted edges (duplicate edges sum).

Strategy (8 NeuronCores, SPMD via run_bass_kernel_spmd):
  - Nodes are dst-sharded: core c owns dst rows [c*6250, (c+1)*6250).
  - Each layer computes z = h·W on local node tiles (TensorE), AllGathers the
    z table (bf16) so every core sees all 50176 (padded) rows, then does the
    sparse aggregation locally:
      * per-edge rows of z are fetched with bulk SWDGE dma_gather (int16
        indices; the table is split in two 25088-row halves so indices fit),
      * segment-sum is a one-hot matmul: S[e, p] = val_e * (dst_rel_e == p)
        built on VectorE (is_equal vs an iota tile), accumulated on TensorE
        into PSUM per 128-node dst tile.
  - Edges are pre-sorted by (core, dst tile, table half) on the host and
    padded per (tile, half) to 9 chunks of 128 edge slots (pad slots gather
    row 0 with val 0, contributing nothing).
  - Layer 3 runs in fp32 with the flipped matmul orientation so the output
    lands directly in [node, 64] layout.

If anything about the input violates the precomputed capacity assumptions
(or no neuron devices are present), falls back to an exact scipy host path.
"""

import os

import numpy as np

# ---------------------------------------------------------------- constants
N_NODES = 50000
N_EDGES = 800000
IN_DIM, HID_DIM, OUT_DIM = 128, 128, 64
NC_ = 8
PER = N_NODES // NC_          # 6250 nodes per core
T = 49                        # dst tiles of 128 nodes per core (49*128 = 6272)
PADN = T * 128                # 6272 padded local rows
TABLE = NC_ * PADN            # 50176 global table rows
LO = TABLE // 2               # 25088: first half of the table (int16-safe)
CPH = 9                       # chunks (of 128 edge slots) per (tile, half)
CAP = CPH * 128               # 1152 slots per (tile, half)
SLOTS_R = T * CAP             # 56448 slots per region (half) per core
COLS_R = SLOTS_R // 128       # 441 gather cols per region
GCALL = 1024                  # max descriptors per dma_gather (ucode ring)
GCOLS = GCALL // 128          # 8 msg cols per full gather call
NPIECE = (COLS_R + GCOLS - 1) // GCOLS  # 56 gather calls per region per layer

# exec time of the last device run (ns), set when GCN_TRACE=1
LAST_EXEC_NS = None
LAST_RESULTS = None


# ---------------------------------------------------------------- host path
def _host_kernel(x, adj_indices, adj_values, W1, b1, W2, b2, W3, b3):
    dst = np.asarray(adj_indices[0], dtype=np.int64)
    src = np.asarray(adj_indices[1], dtype=np.int64)
    vals = np.asarray(adj_values, dtype=np.float32)
    from scipy.sparse import csr_matrix

    A = csr_matrix((vals, (dst, src)), shape=(N_NODES, N_NODES))
    h = np.asarray(A @ np.asarray(x, dtype=np.float32)) @ W1 + b1
    h = np.asarray(A @ h) @ W2 + b2
    return (np.asarray(A @ h) @ W3 + b3).astype(np.float32)


# ----------------------------------------------------------- host-side prep
def _prep_edges(adj_indices, adj_values):
    """Sort/pad edges into the per-core slot layout.

    Returns per-core packed arrays or None if capacities are exceeded:
      idx  [8, 2, 128, SLOTS_R//16] int16  (dma_gather wrapped layout)
      drel [8, 2, 128, COLS_R] float32     (dst one-hot position, 0..127)
      val  [8, 2, 128, COLS_R] float32
    """
    dst = np.asarray(adj_indices[0], dtype=np.int64)
    src = np.asarray(adj_indices[1], dtype=np.int64)
    vals = np.asarray(adj_values, dtype=np.float32)
    if dst.min() < 0 or dst.max() >= N_NODES or src.min() < 0 or src.max() >= N_NODES:
        return None

    core = dst // PER
    dl = dst - core * PER
    tl = dl >> 7
    drel = (dl & 127).astype(np.float32)
    core_s = src // PER
    tp = core_s * PADN + (src - core_s * PER)
    hi = (tp >= LO).astype(np.int64)
    idxv = np.where(hi == 1, tp - LO, tp).astype(np.int64)

    key = (core * T + tl) * 2 + hi
    counts = np.bincount(key, minlength=NC_ * T * 2)
    if counts.max() > CAP:
        return None
    order = np.argsort(key, kind="stable")
    starts = np.zeros_like(counts)
    starts[1:] = np.cumsum(counts)[:-1]
    g = key[order]
    rank = np.arange(len(dst)) - starts[g]
    c_ = g // (T * 2)
    rem = g - c_ * (T * 2)
    t_ = rem >> 1
    h_ = rem & 1
    slot = t_ * CAP + rank

    idx_arr = np.zeros((NC_, 2, SLOTS_R), np.int16)
    dr_arr = np.zeros((NC_, 2, SLOTS_R), np.float32)
    va_arr = np.zeros((NC_, 2, SLOTS_R), np.float32)
    idx_arr[c_, h_, slot] = idxv[order].astype(np.int16)
    dr_arr[c_, h_, slot] = drel[order]
    va_arr[c_, h_, slot] = vals[order]

    # dma_gather index layout: slot i -> [partition i%16 (replicated to 128), col i//16]
    idx_w = idx_arr.reshape(NC_, 2, SLOTS_R // 16, 16).transpose(0, 1, 3, 2)
    idx_buf = np.tile(idx_w, (1, 1, 8, 1))  # [8, 2, 128, 3528]
    # drel/val layout: slot i -> [partition i%128, col i//128]
    dr_buf = np.ascontiguousarray(dr_arr.reshape(NC_, 2, COLS_R, 128).transpose(0, 1, 3, 2))
    va_buf = np.ascontiguousarray(va_arr.reshape(NC_, 2, COLS_R, 128).transpose(0, 1, 3, 2))
    return np.ascontiguousarray(idx_buf), dr_buf, va_buf


# ------------------------------------------------------------ device kernel
def _build_nc():
    import concourse.bacc as bacc
    import concourse.bass as bass
    import concourse.mybir as mybir
    import concourse.tile as tile

    BF16 = mybir.dt.bfloat16
    F32 = mybir.dt.float32
    I16 = mybir.dt.int16
    ts = bass.ts

    nc = bacc.Bacc("TRN2", target_bir_lowering=False, debug=False,
                   enable_asserts=False, num_devices=NC_)

    xT_in = nc.dram_tensor("xT", [128, PADN], BF16, kind="ExternalInput")
    idxA_in = nc.dram_tensor("idxA", [128, SLOTS_R // 16], I16, kind="ExternalInput")
    idxB_in = nc.dram_tensor("idxB", [128, SLOTS_R // 16], I16, kind="ExternalInput")
    drA_in = nc.dram_tensor("drA", [128, COLS_R], BF16, kind="ExternalInput")
    drB_in = nc.dram_tensor("drB", [128, COLS_R], BF16, kind="ExternalInput")
    vaA_in = nc.dram_tensor("vaA", [128, COLS_R], BF16, kind="ExternalInput")
    vaB_in = nc.dram_tensor("vaB", [128, COLS_R], BF16, kind="ExternalInput")
    drA32_in = nc.dram_tensor("drA32", [128, COLS_R], F32, kind="ExternalInput")
    drB32_in = nc.dram_tensor("drB32", [128, COLS_R], F32, kind="ExternalInput")
    vaA32_in = nc.dram_tensor("vaA32", [128, COLS_R], F32, kind="ExternalInput")
    vaB32_in = nc.dram_tensor("vaB32", [128, COLS_R], F32, kind="ExternalInput")
    w1_in = nc.dram_tensor("w1", [128, 128], BF16, kind="ExternalInput")
    w2_in = nc.dram_tensor("w2", [128, 128], BF16, kind="ExternalInput")
    w3_in = nc.dram_tensor("w3", [128, OUT_DIM], BF16, kind="ExternalInput")
    b1_in = nc.dram_tensor("b1", [128, 1], F32, kind="ExternalInput")
    b2_in = nc.dram_tensor("b2", [128, 1], F32, kind="ExternalInput")
    b3b_in = nc.dram_tensor("b3b", [128, OUT_DIM], F32, kind="ExternalInput")
    iota_in = nc.dram_tensor("iota", [128, 128], BF16, kind="ExternalInput")
    iota32_in = nc.dram_tensor("iota32", [128, 128], F32, kind="ExternalInput")
    out = nc.dram_tensor("out", [PADN, OUT_DIM], F32, kind="ExternalOutput")

    groups = [list(range(NC_))]

    with tile.TileContext(nc) as tc:
        with (
            tc.tile_pool(name="const", bufs=1) as cp,
            tc.tile_pool(name="work", bufs=2) as wp,
            tc.tile_pool(name="ms", bufs=4) as mp,
            tc.tile_pool(name="psum", bufs=2, space="PSUM") as pp,
            tc.tile_pool(name="dram", bufs=1, space="DRAM") as dp,
        ):
            def load(name, inp, shape, dt):
                t = cp.tile(shape, dt, tag=name)
                nc.sync.dma_start(out=t[:], in_=inp[:, :])
                return t

            xT = load("xT", xT_in, [128, PADN], BF16)
            idxA = load("idxA", idxA_in, [128, SLOTS_R // 16], I16)
            idxB = load("idxB", idxB_in, [128, SLOTS_R // 16], I16)
            drA = load("drA", drA_in, [128, COLS_R], BF16)
            drB = load("drB", drB_in, [128, COLS_R], BF16)
            vaA = load("vaA", vaA_in, [128, COLS_R], BF16)
            vaB = load("vaB", vaB_in, [128, COLS_R], BF16)
            drA32 = load("drA32", drA32_in, [128, COLS_R], F32)
            drB32 = load("drB32", drB32_in, [128, COLS_R], F32)
            vaA32 = load("vaA32", vaA32_in, [128, COLS_R], F32)
            vaB32 = load("vaB32", vaB32_in, [128, COLS_R], F32)
            w1 = load("w1", w1_in, [128, 128], BF16)
            w2 = load("w2", w2_in, [128, 128], BF16)
            w3 = load("w3", w3_in, [128, OUT_DIM], BF16)
            b1 = load("b1", b1_in, [128, 1], F32)
            b2 = load("b2", b2_in, [128, 1], F32)
            b3b = load("b3b", b3b_in, [128, OUT_DIM], F32)
            iota = load("iota", iota_in, [128, 128], BF16)
            iota32 = load("iota32", iota32_in, [128, 128], F32)

            z1l = dp.tile([PADN, 128], BF16)
            zf1 = dp.tile([TABLE, 128], BF16, addr_space="Shared")
            z2l = dp.tile([PADN, 128], BF16)
            zf2 = dp.tile([TABLE, 128], BF16, addr_space="Shared")
            z3l = dp.tile([PADN, OUT_DIM], F32)
            zf3 = dp.tile([TABLE, OUT_DIM], F32, addr_space="Shared")

            def bc3(ap2d, t, n):
                # [128, COLS_R] slice for tile t -> [128, CPH, n] broadcast
                return ap2d[:, t * CPH:(t + 1) * CPH].to_broadcast([128, CPH, n])

            def iota3(it, n):
                a = it[:]
                return bass.AP(a.tensor, a.offset, [a.ap[0], [0, CPH], [1, n]])

            # ---- dense layer 1: z1 = x @ W1 (per local tile)
            for t in range(T):
                pz = pp.tile([128, 128], F32, tag="pz", space="PSUM")
                nc.tensor.matmul(out=pz[:], lhsT=xT[:, ts(t, 128)], rhs=w1[:],
                                 start=True, stop=True)
                zt = wp.tile([128, 128], BF16, tag="zt")
                nc.scalar.copy(zt[:], pz[:])
                nc.sync.dma_start(out=z1l[ts(t, 128), :], in_=zt[:])
            nc.gpsimd.collective_compute(
                "AllGather", mybir.AluOpType.bypass, replica_groups=groups,
                ins=[z1l.opt()], outs=[zf1.opt()])

            # ---- middle layers: h = A·z + b ; z_next = h @ W_next
            def make_gather_mgr(zf, idx_t, lo_region, elem, dt):
                state = {"next": 0, "pieces": {}}

                def ensure(p):
                    while state["next"] <= p:
                        q = state["next"]
                        cols = min(GCOLS, COLS_R - q * GCOLS)
                        nidx = cols * 128
                        ms = mp.tile([128, cols, elem], dt,
                                     tag="msA" if lo_region else "msB")
                        src_ap = zf[0:LO, :] if lo_region else zf[LO:TABLE, :]
                        nc.gpsimd.dma_gather(
                            out_ap=ms[:], in_ap=src_ap,
                            idxs_ap=idx_t[:, q * (GCALL // 16):
                                          q * (GCALL // 16) + nidx // 16],
                            num_idxs=nidx, num_idxs_reg=nidx,
                            elem_size=elem, queue_num=0)
                        state["pieces"][q] = ms
                        state["next"] = q + 1

                def col(c):
                    return state["pieces"][c // GCOLS][:, c % GCOLS, :]

                def prefetch(t):
                    ensure(min((t * CPH + CPH - 1) // GCOLS, NPIECE - 1))

                return prefetch, col

            def spmm_mid(zf, bias, w_next, z_next):
                pfA, colA = make_gather_mgr(zf, idxA, True, 128, BF16)
                pfB, colB = make_gather_mgr(zf, idxB, False, 128, BF16)
                for t in range(T):
                    pfA(t)
                    pfB(t)
                    SA = wp.tile([128, CPH, 128], BF16, tag="SA")
                    nc.vector.tensor_tensor(out=SA[:], in0=bc3(drA, t, 128),
                                            in1=iota3(iota, 128),
                                            op=mybir.AluOpType.is_equal)
                    nc.vector.tensor_tensor(out=SA[:], in0=SA[:],
                                            in1=bc3(vaA, t, 128),
                                            op=mybir.AluOpType.mult)
                    SB = wp.tile([128, CPH, 128], BF16, tag="SB")
                    nc.vector.tensor_tensor(out=SB[:], in0=bc3(drB, t, 128),
                                            in1=iota3(iota, 128),
                                            op=mybir.AluOpType.is_equal)
                    nc.vector.tensor_tensor(out=SB[:], in0=SB[:],
                                            in1=bc3(vaB, t, 128),
                                            op=mybir.AluOpType.mult)
                    pt = pp.tile([128, 128], F32, tag="pt", space="PSUM")
                    for k in range(CPH):
                        nc.tensor.matmul(out=pt[:], lhsT=colA(t * CPH + k),
                                         rhs=SA[:, k, :], start=(k == 0), stop=False)
                    for k in range(CPH):
                        nc.tensor.matmul(out=pt[:], lhsT=colB(t * CPH + k),
                                         rhs=SB[:, k, :], start=False,
                                         stop=(k == CPH - 1))
                    hT = wp.tile([128, 128], BF16, tag="hT")
                    nc.vector.tensor_tensor(out=hT[:], in0=pt[:],
                                            in1=bias[:, 0:1].to_broadcast([128, 128]),
                                            op=mybir.AluOpType.add)
                    ncols = w_next.shape[-1]
                    pz = pp.tile([128, 128], F32, tag="pz", space="PSUM")
                    nc.tensor.matmul(out=pz[:, :ncols], lhsT=hT[:], rhs=w_next[:],
                                     start=True, stop=True)
                    if ncols == 128:
                        zt = wp.tile([128, 128], BF16, tag="zt")
                        nc.scalar.copy(zt[:], pz[:])
                        nc.sync.dma_start(out=z_next[ts(t, 128), :], in_=zt[:])
                    else:
                        zt = wp.tile([128, ncols], F32, tag="zt32")
                        nc.scalar.copy(zt[:], pz[:, :ncols])
                        nc.sync.dma_start(out=z_next[ts(t, 128), :], in_=zt[:])

            spmm_mid(zf1, b1, w2, z2l)
            nc.gpsimd.collective_compute(
                "AllGather", mybir.AluOpType.bypass, replica_groups=groups,
                ins=[z2l.opt()], outs=[zf2.opt()])
            spmm_mid(zf2, b2, w3, z3l)
            nc.gpsimd.collective_compute(
                "AllGather", mybir.AluOpType.bypass, replica_groups=groups,
                ins=[z3l.opt()], outs=[zf3.opt()])

            # ---- final layer: out = A·z3 + b3 (fp32, direct [node, 64] layout)
            pfA, colA = make_gather_mgr(zf3, idxA, True, OUT_DIM, F32)
            pfB, colB = make_gather_mgr(zf3, idxB, False, OUT_DIM, F32)
            for t in range(T):
                pfA(t)
                pfB(t)
                SA = wp.tile([128, CPH, 128], F32, tag="SA32")
                nc.vector.tensor_tensor(out=SA[:], in0=bc3(drA32, t, 128),
                                        in1=iota3(iota32, 128),
                                        op=mybir.AluOpType.is_equal)
                nc.vector.tensor_tensor(out=SA[:], in0=SA[:],
                                        in1=bc3(vaA32, t, 128),
                                        op=mybir.AluOpType.mult)
                SB = wp.tile([128, CPH, 128], F32, tag="SB32")
                nc.vector.tensor_tensor(out=SB[:], in0=bc3(drB32, t, 128),
                                        in1=iota3(iota32, 128),
                                        op=mybir.AluOpType.is_equal)
                nc.vector.tensor_tensor(out=SB[:], in0=SB[:],
                                        in1=bc3(vaB32, t, 128),
                                        op=mybir.AluOpType.mult)
                pt = pp.tile([128, OUT_DIM], F32, tag="pt", space="PSUM")
                for k in range(CPH):
                    nc.tensor.matmul(out=pt[:], lhsT=SA[:, k, :],
                                     rhs=colA(t * CPH + k), start=(k == 0), stop=False)
                for k in range(CPH):
                    nc.tensor.matmul(out=pt[:], lhsT=SB[:, k, :],
                                     rhs=colB(t * CPH + k), start=False,
                                     stop=(k == CPH - 1))
                ot = wp.tile([128, OUT_DIM], F32, tag="ot")
                nc.vector.tensor_tensor(out=ot[:], in0=pt[:], in1=b3b[:, :],
                                        op=mybir.AluOpType.add)
                nc.sync.dma_start(out=out[ts(t, 128), :], in_=ot[:])

    nc.compile()
    return nc


_NC_CACHE = None
_EXEC_CACHE = None
_LAST_IN_MAPS = None


def _get_exec():
    """Build (once) a cached jitted SPMD executor for the bass module.

    Mirrors concourse.bass2jax.run_bass_via_pjrt's multi-core branch, but
    keeps the jitted callable so repeat executions skip jit rebuild.
    """
    global _EXEC_CACHE
    if _EXEC_CACHE is not None:
        return _EXEC_CACHE
    import jax
    import concourse.mybir as mybir
    from concourse import bass2jax
    from jax.experimental.shard_map import shard_map
    from jax.sharding import Mesh, PartitionSpec

    nc = _NC_CACHE
    bass2jax.install_neuronx_cc_hook()
    partition_name = nc.partition_id_tensor.name if nc.partition_id_tensor else None
    in_names, out_names, out_avals = [], [], []
    for alloc in nc.m.functions[0].allocations:
        if not isinstance(alloc, mybir.MemoryLocationSet):
            continue
        name = alloc.memorylocations[0].name
        if alloc.kind == "ExternalInput":
            if name != partition_name:
                in_names.append(name)
        elif alloc.kind == "ExternalOutput":
            out_names.append(name)
            out_avals.append(jax.core.ShapedArray(
                tuple(alloc.tensor_shape), mybir.dt.np(alloc.dtype)))
    n_params = len(in_names)
    all_in_names = list(in_names) + list(out_names)
    if partition_name is not None:
        all_in_names.append(partition_name)
    donate = tuple(range(n_params, n_params + len(out_names)))

    def _body(*args):
        operands = list(args)
        if partition_name is not None:
            operands.append(bass2jax.partition_id_tensor())
        outs = bass2jax._bass_exec_p.bind(
            *operands,
            out_avals=tuple(out_avals),
            in_names=tuple(all_in_names),
            out_names=tuple(out_names),
            lowering_input_output_aliases=(),
            sim_require_finite=True,
            sim_require_nnan=True,
            nc=nc,
        )
        return tuple(outs)

    devices = jax.devices()[:NC_]
    mesh = Mesh(np.asarray(devices), ("core",))
    n_out = len(out_names)
    sharded = jax.jit(
        shard_map(_body, mesh=mesh,
                  in_specs=(PartitionSpec("core"),) * (n_params + n_out),
                  out_specs=(PartitionSpec("core"),) * n_out,
                  check_rep=False),
        donate_argnums=donate, keep_unused=True)
    _EXEC_CACHE = (sharded, in_names, out_names, out_avals)
    return _EXEC_CACHE


def _exec_in_maps(in_maps):
    sharded, in_names, out_names, out_avals = _get_exec()
    concat_in = [
        np.concatenate([np.asarray(in_maps[c][k]) for c in range(NC_)], axis=0)
        for k in in_names
    ]
    concat_zeros = [
        np.zeros((NC_ * a.shape[0], *a.shape[1:]), a.dtype) for a in out_avals
    ]
    out_arrs = sharded(*concat_in, *concat_zeros)
    return [
        {k: np.asarray(out_arrs[i]).reshape(NC_, *out_avals[i].shape)[c]
         for i, k in enumerate(out_names)}
        for c in range(NC_)
    ]


def bench(n=5):
    """Re-run the last device execution n times; returns wall seconds per run
    (device-dispatch + execute + output download, inputs re-uploaded)."""
    import time
    assert _LAST_IN_MAPS is not None
    times = []
    for _ in range(n):
        t0 = time.perf_counter()
        _exec_in_maps(_LAST_IN_MAPS)
        times.append(time.perf_counter() - t0)
    return times


def _device_kernel(x, adj_indices, adj_values, W1, b1, W2, b2, W3, b3):
    global _NC_CACHE, LAST_EXEC_NS, LAST_RESULTS, _LAST_IN_MAPS
    import ml_dtypes

    prep = _prep_edges(adj_indices, adj_values)
    if prep is None:
        raise ValueError("edge layout exceeds padded capacity")
    idx_buf, dr_buf, va_buf = prep

    bf16 = ml_dtypes.bfloat16
    x = np.asarray(x, dtype=np.float32)
    W1 = np.asarray(W1, np.float32)
    W2 = np.asarray(W2, np.float32)
    W3 = np.asarray(W3, np.float32)
    iota_np = np.broadcast_to(np.arange(128, dtype=np.float32), (128, 128))
    b3b_np = np.ascontiguousarray(
        np.broadcast_to(np.asarray(b3, np.float32)[None, :], (128, OUT_DIM)))

    shared = {
        "w1": W1.astype(bf16),
        "w2": W2.astype(bf16),
        "w3": W3.astype(bf16),
        "b1": np.asarray(b1, np.float32).reshape(128, 1),
        "b2": np.asarray(b2, np.float32).reshape(128, 1),
        "b3b": b3b_np,
        "iota": np.ascontiguousarray(iota_np.astype(bf16)),
        "iota32": np.ascontiguousarray(iota_np.astype(np.float32)),
    }
    in_maps = []
    for c in range(NC_):
        xs = np.zeros((128, PADN), np.float32)
        xs[:, :PER] = x[c * PER:(c + 1) * PER].T
        m = {
            "xT": xs.astype(bf16),
            "idxA": idx_buf[c, 0],
            "idxB": idx_buf[c, 1],
            "drA": dr_buf[c, 0].astype(bf16),
            "drB": dr_buf[c, 1].astype(bf16),
            "vaA": va_buf[c, 0].astype(bf16),
            "vaB": va_buf[c, 1].astype(bf16),
            "drA32": dr_buf[c, 0],
            "drB32": dr_buf[c, 1],
            "vaA32": va_buf[c, 0],
            "vaB32": va_buf[c, 1],
        }
        m.update(shared)
        in_maps.append(m)

    if _NC_CACHE is None:
        _NC_CACHE = _build_nc()

    _LAST_IN_MAPS = in_maps
    results = _exec_in_maps(in_maps)
    LAST_RESULTS = results
    outp = np.concatenate([results[c]["out"][:PER] for c in range(NC_)], axis=0)
    return np.ascontiguousarray(outp.astype(np.float32))


def kernel(x, adj_indices, adj_values, W1, b1, W2, b2, W3, b3):
    if os.environ.get("GCN_FORCE_HOST"):
        return _host_kernel(x, adj_indices, adj_values, W1, b1, W2, b2, W3, b3)
    try:
        return _device_kernel(x, adj_indices, adj_values, W1, b1, W2, b2, W3, b3)
    except Exception:
        if os.environ.get("GCN_NO_FALLBACK"):
            raise
        return _host_kernel(x, adj_indices, adj_values, W1, b1, W2, b2, W3, b3)
